# revision 26
# baseline (speedup 1.0000x reference)
"""Single-head causal attention on 8 trn2 NeuronCores (one batch element per core).

Problem: x [8, 2048, 1024], Wq/Wk/Wv [1024, 64] -> out [8, 2048, 64]
  q = x@Wq; k = x@Wk; v = x@Wv; out = causal_softmax(q k^T / sqrt(64)) @ v

Strategy (per core, batch-parallel across the 8 cores):
  - Host pre-transposes each core's x to x^T [E, S] so the QKV projections can
    contract over E with E on SBUF partitions (no on-chip transpose of x).
  - Projections on the PE as fp32r matmuls: Q^T and K^T are produced
    *duplicated* across partition halves (lhsT = [Wq|Wq]) so the score
    matmuls can be packed two-at-a-time into disjoint PE row groups.
  - Scores are computed transposed (P^T[kv, q]) so that softmax normalization
    can ride the PV matmul: V is augmented with a ones column, making row 64
    of the PV output the softmax denominator. No max-subtraction is needed
    (scores are O(1) by construction; exp cannot overflow fp32).
  - exp on ScalarE straight out of PSUM with the 1/sqrt(D) scale folded in.
  - Causal masking is a multiplicative 0/1 mask applied after exp, only on
    diagonal tiles, sliced from one precomputed [128, 1024] step mask.
  - PV accumulates out^T in PSUM; a PE transpose brings it back to natural
    layout where the per-query normalizer lands on the partition dim, so the
    divide is a reciprocal + per-partition tensor_scalar multiply.
"""

import numpy as np

import concourse.bass as bass
import concourse.mybir as mybir
import concourse.tile as tile
from concourse.vector_clock import ScopedClock

S = 2048  # sequence length
E = 1024  # embed dim
D = 64    # head size
B = 8     # batch == number of cores
P = 128   # SBUF partitions
SBLK = 512         # q-block / s-block width (max fp32 matmul moving dim)
EC = E // P        # 8 contraction chunks
NSB = S // SBLK    # 4 s-blocks
NJT = S // P       # 16 kv tiles

f32 = mybir.dt.float32
f32r = mybir.dt.float32r
f16 = mybir.dt.float16
MMDT = f16          # dtype of all large-matmul operands
MMNP = np.float16   # matching numpy dtype for host-side prep
AF = mybir.ActivationFunctionType

_PATCHED = False


def _patch_tile_drain():
    """The walrus build in this container rejects instructions carrying more
    than one sem wait on the Tile exit Drain. Split the waits across a chain
    of drains, one wait each."""
    global _PATCHED
    if _PATCHED:
        return
    _PATCHED = True

    def _drain_and_barrier(self, tick_clock, wait_clock):
        drain_inst = self.nc.sync.drain()
        wait_clock.add_sem_waits(
            drain_inst.ins, ScopedClock({None: tick_clock.global_clock})
        )
        ins = drain_inst.ins
        si = ins.sync_info
        if si is not None and si.on_wait is not None and len(si.on_wait) > 1:
            waits = list(si.on_wait)
            ins.sync_info = mybir.SyncInfo(
                on_wait=[waits[0]], on_update=list(si.on_update or [])
            )
            for w in waits[1:]:
                d2 = self.nc.sync.drain()
                d2.ins.sync_info = mybir.SyncInfo(on_wait=[w], on_update=[])
        self.nc.all_engine_barrier()
        assert self.sems is not None
        popped = self.nc._tile_sem_poison_stack.pop()
        assert popped is self._sem_poison
        self.nc.clear_and_free_semaphores(list(self.sems.allocated().values()))
        self.nc.all_engine_barrier()

    tile.TileContext._drain_and_barrier = _drain_and_barrier


def _split_multiwaits(nc):
    """This container's walrus rejects instructions carrying more than one
    sem wait (setupSyncWait: 'Too many sync wait commands'). Hoist all but
    the last wait of every instruction onto same-engine NoOps placed
    immediately before it — the engine sequencer processes them in order,
    which is semantically identical."""
    ctr = 0
    for f in nc.m.functions:
        for bb in f.blocks:
            out = []
            changed = False
            for inst in bb.instructions:
                si = inst.sync_info
                if si is not None and si.on_wait is not None and len(si.on_wait) > 1:
                    waits = list(si.on_wait)
                    for w in waits[:-1]:
                        nop = mybir.InstNoOp(name=f"I-waitsplit-{ctr}")
                        ctr += 1
                        nop.engine = inst.engine
                        nop.sync_info = mybir.SyncInfo(on_wait=[w], on_update=[])
                        out.append(nop)
                    inst.sync_info = mybir.SyncInfo(
                        on_wait=[waits[-1]], on_update=list(si.on_update or [])
                    )
                    changed = True
                out.append(inst)
            if changed:
                bb.instructions = out


def _attention(ctx, tc, xt, wqk, wv, y):
    nc = tc.nc
    scale = 1.0 / np.sqrt(D)

    persist = ctx.enter_context(tc.tile_pool(name="persist", bufs=1))
    xpool = ctx.enter_context(tc.tile_pool(name="xts", bufs=1))
    ppool = ctx.enter_context(tc.tile_pool(name="pp", bufs=6))
    opool = ctx.enter_context(tc.tile_pool(name="ot", bufs=2))
    rpool = ctx.enter_context(tc.tile_pool(name="rec", bufs=8))
    # PSUM budget (8 banks): psproj 1 (QK/V serialize naturally through the
    # proj step order; the warmup target shares the slot) + psscore 2x2 +
    # pspv 1 + pstr 2 (double-buffered transpose targets so the
    # PE-transpose -> DVE-copy chain pipelines instead of round-tripping).
    psproj = ctx.enter_context(tc.tile_pool(name="psproj", bufs=1, space="PSUM"))
    psscore = ctx.enter_context(tc.tile_pool(name="psscore", bufs=2, space="PSUM"))
    pspv = ctx.enter_context(tc.tile_pool(name="pspv", bufs=1, space="PSUM"))
    pstr = ctx.enter_context(tc.tile_pool(name="pstr", bufs=2, space="PSUM"))

    # ---- weights (dual queue: wqk on sync, wv on scalar) ----------------
    wqk_sb = persist.tile([P, EC, 2 * D], MMDT, tag="wqk")  # [Wq|Wk] packed
    wv_sb = persist.tile([P, EC, D], MMDT, tag="wv")
    nc.sync.dma_start(wqk_sb[:], wqk.rearrange("(c p) m -> p c m", p=P))
    nc.scalar.dma_start(wv_sb[:], wv.rearrange("(c p) m -> p c m", p=P))

    # ---- PE warm-up: keep HAM busy while the input streams in -----------
    warm_in = persist.tile([P, SBLK], MMDT, tag="warm")
    nc.vector.memset(warm_in[:], 0.25)
    # dummy activation: pull the ~1.3us ACT table load into the DMA phase
    warm_act = rpool.tile([P, 8], f32, tag="warmact")
    nc.scalar.activation(warm_act[:], warm_in[:, :8], AF.Exp, scale=1.0)
    wt = psproj.tile([P, SBLK], f32, tag="proj")
    for _ in range(8):
        nc.tensor.matmul(wt[:], warm_in[:, :P], warm_in[:], start=True, stop=True)

    # ---- constants -------------------------------------------------------
    ident = persist.tile([P, P], f32, tag="ident")
    nc.gpsimd.memset(ident[:], 0.0)
    nc.gpsimd.affine_select(
        out=ident[:], in_=ident[:],
        compare_op=mybir.AluOpType.not_equal, fill=1.0,
        base=0, pattern=[[-1, P]], channel_multiplier=1,
    )
    ident16 = persist.tile([P, P], MMDT, tag="ident16")
    nc.vector.tensor_copy(ident16[:], ident[:])

    # causal step mask: maskW[jj, c] = 1 iff c >= jj + SBLK
    maskW = persist.tile([P, 2 * SBLK], f32, tag="maskw")
    nc.gpsimd.memset(maskW[:], 1.0)
    nc.gpsimd.affine_select(
        out=maskW[:], in_=maskW[:],
        compare_op=mybir.AluOpType.is_ge, fill=0.0,
        base=-SBLK, pattern=[[1, 2 * SBLK]], channel_multiplier=-1,
    )
    mask16 = persist.tile([P, 2 * SBLK], MMDT, tag="mask16")
    nc.vector.tensor_copy(mask16[:], maskW[:])
    # pair-merged mask operands: one [P, 2, w] tile per diagonal pair so a
    # single tensor_mul masks both members of the pair
    maskP0 = persist.tile([P, 2, SBLK], MMDT, tag="maskp0")
    nc.vector.tensor_copy(maskP0[:, 0, :], mask16[:, SBLK : 2 * SBLK])
    nc.vector.tensor_copy(maskP0[:, 1, :], mask16[:, SBLK - P : 2 * SBLK - P])
    # member k covers tile t=2+k narrowed to cols [2P, SBLK): slice
    # mask16[:, SBLK - t*P + 2P : 2*SBLK - t*P]
    maskP1 = persist.tile([P, 2, SBLK - 2 * P], MMDT, tag="maskp1")
    nc.vector.tensor_copy(
        maskP1[:, 0, :], mask16[:, SBLK : 2 * SBLK - 2 * P]
    )
    nc.vector.tensor_copy(
        maskP1[:, 1, :], mask16[:, SBLK - P : 2 * SBLK - 3 * P]
    )

    # ---- persistent activations -----------------------------------------
    # qk: rows 0:64 = Q^T, rows 64:128 = K^T (straight from packed psum)
    qk = persist.tile([P, S], MMDT, tag="qk")
    # partition-shifted copies (SBUF->SBUF DMA): K^T at rows 0:64, Q^T at 64:128
    kTlo = persist.tile([D, S], MMDT, tag="ktlo")
    qThi = persist.tile([P, S], MMDT, tag="qthi")  # rows 64:128 used
    vT = persist.tile([D, S], MMDT, tag="vt")
    vAug = persist.tile([P, NJT, 2 * D], MMDT, tag="vaug")
    yT = persist.tile([D, S], f32, tag="ytout")
    ones_f32 = persist.tile([P, NJT, D], f32, tag="ones")
    nc.vector.memset(ones_f32[:], 1.0)
    nc.vector.tensor_copy(vAug[:, :, D:], ones_f32[:])

    # ---- stream x^T block-by-block -------------------------------------
    # Static DMAs land on separate logical queues and would all stream
    # CONCURRENTLY from t=0, sharing bandwidth fairly and starving block 0.
    # Gate block b's load on earlier compute via tiny WAW writes into the
    # destination tile: the DMA then fires only once the gate ran, keeping
    # HBM focused on the block the pipeline needs next.
    xt_r = xt.rearrange("(c p) s -> p c s", p=P)
    xts = []
    for b in range(NSB):
        xts_b = xpool.tile([P, EC, SBLK], MMDT, tag=f"xts{b}", name=f"xts{b}")
        xts.append(xts_b)

    def stream_x(b):
        sl_b = slice(b * SBLK, (b + 1) * SBLK)
        nc.sync.dma_start(xts[b][:, : EC // 2, :], xt_r[:, : EC // 2, sl_b])
        nc.scalar.dma_start(xts[b][:, EC // 2 :, :], xt_r[:, EC // 2 :, sl_b])

    def gate_x(b, src):
        nc.vector.tensor_copy(xts[b][0:1, 0, 0:1], src)
        nc.vector.tensor_copy(xts[b][0:1, EC // 2, 0:1], src)

    stream_x(0)
    gate_x(1, wt[0:1, 0:1])  # x1 fires once the warmup matmuls finish
    stream_x(1)

    def qk_chunks(b):
        """Emit-steps for s-block b's Q/K projection."""
        sl = slice(b * SBLK, (b + 1) * SBLK)
        steps = []
        psQK = psproj.tile([P, SBLK], f32, tag="proj")

        def qk_mms(e0, psQK=psQK):
            for e in (e0, e0 + 1):
                nc.tensor.matmul(
                    psQK[:], wqk_sb[:, e, :], xts[b][:, e, :],
                    start=(e == 0), stop=(e == EC - 1),
                )

        for e0 in range(0, EC, 2):
            steps.append(lambda e0=e0: qk_mms(e0))

        def qk_out():
            nc.vector.tensor_copy(qk[:, sl], psQK[:])

        # partition-shifted copies on DVE (APs with equal partition counts
        # may live at different base partitions); f16 source for 2x rate
        def qk_shift_k():
            nc.vector.tensor_copy(kTlo[:, sl], qk[D:P, sl])

        def qk_shift_q():
            nc.vector.tensor_copy(qThi[D:P, sl], qk[:D, sl])

        steps.append(qk_out)
        steps.append(qk_shift_k)
        steps.append(qk_shift_q)
        if 1 <= b < NSB - 1:
            def gate_next():
                # release block b+1's x stream once this block's projection
                # is through the PE
                gate_x(b + 1, psQK[0:1, 0:1])
                stream_x(b + 1)

            steps.append(gate_next)
        return steps

    def v_chunks(b):
        """Emit-steps for s-block b's V projection: chunk pairs run
        concurrently as column-tiled matmuls (PE col groups 0:64 / 64:128),
        halving PE time; the halves are summed by the psum->sbuf add."""
        sl = slice(b * SBLK, (b + 1) * SBLK)
        steps = []
        psV = psproj.tile([P, SBLK], f32, tag="proj")

        def v_mms(e0, psV=psV):
            for k, e in enumerate((e0, e0 + 1)):
                nc.tensor.matmul(
                    psV[k * D : (k + 1) * D, :], wv_sb[:, e, :], xts[b][:, e, :],
                    start=(e0 == 0), stop=(e0 == EC - 2),
                )

        for e0 in range(0, EC, 2):
            steps.append(lambda e0=e0: v_mms(e0))

        def v_out():
            # DVE may read only one PSUM operand: stage the odd-chunk half
            # in SBUF, then add it to the even-chunk half.
            vhalf = rpool.tile([D, SBLK], MMDT, tag="vhalf")
            nc.vector.tensor_copy(vhalf[:], psV[D:, :])
            nc.vector.tensor_add(vT[:, sl], psV[:D, :], vhalf[:])

        steps.append(v_out)

        def v_tr(t):
            j = 4 * b + t
            psv_t = pstr.tile([P, D], MMDT, tag="tr")
            nc.tensor.transpose(
                psv_t[:], vT[:, j * P : (j + 1) * P], ident16[:D, :D]
            )
            nc.vector.tensor_copy(vAug[:, j, :D], psv_t[:])

        for t in range(4):
            steps.append(lambda t=t: v_tr(t))
        return steps

    def proj_chunks(b):
        return qk_chunks(b) + v_chunks(b)

    def scores_pair_for(b, pi):
        """Scores for kv tiles (2pi, 2pi+1) against q-block b. The second
        diagonal pair (tiles t=2,3) narrows BOTH members to cols >= 2P so
        its exp and mask each collapse to one strided instruction; all other
        pairs run full width. Narrowed matmuls still cover the full exp-read
        region, so exp never reads stale psum."""
        j0, j1 = 2 * pi, 2 * pi + 1
        o = 2 * P if (j0 - 4 * b) == 2 else 0
        ps = psscore.tile([P, 2, SBLK], f32, tag="score")
        q = slice(b * SBLK + o, (b + 1) * SBLK)
        # two PE row-groups: rows 0:64 (kTlo/qk) and 64:128 (qk/qThi)
        nc.tensor.matmul(
            ps[:, 0, o:], kTlo[:, j0 * P : (j0 + 1) * P], qk[:D, q],
        )
        nc.tensor.matmul(
            ps[:, 1, o:], qk[D:P, j1 * P : (j1 + 1) * P], qThi[D:P, q],
        )
        return (j0, j1, ps, o)

    def attn(b, bg=(), inflight=None):
        bg = list(bg)
        nj = 4 * b + 4
        psO = pspv.tile([P, SBLK], f32, tag="pv")
        npairs = nj // 2
        if inflight is None:
            inflight = scores_pair_for(b, 0)
        nxt = None
        for pi in range(npairs):
            j0, j1, ps, off = inflight
            t0 = j0 - 4 * b
            pt = ppool.tile([P, 2, SBLK], MMDT, tag="pt")
            # one exp per pair: contiguous when full-width, strided when the
            # diagonal pair is narrowed (both members share the offset)
            nc.scalar.activation(
                pt[:, :, off:], ps[:, :, off:], AF.Exp, scale=float(scale)
            )
            if t0 >= 0:
                # one mask multiply per diagonal pair via the pair-mask tiles
                mk = maskP0 if t0 == 0 else maskP1
                nc.gpsimd.tensor_mul(pt[:, :, off:], pt[:, :, off:], mk[:])
            if pi + 1 < npairs:
                inflight = scores_pair_for(b, pi + 1)
            elif b + 1 < NSB:
                # cross-block prefetch: next block's first score pair goes
                # out before this block's last PV, so ScalarE rolls into the
                # next block's exps with no boundary bubble. Requires bg
                # (incl. next block's qk_out/shifts) fully drained by now.
                nxt = scores_pair_for(b + 1, 0)
            # independent next-block projection work keeps the PE busy
            # while this pair's exp runs on ScalarE; drain everything
            # before the last pair so the prefetched scores see their data
            if pi < npairs - 1:
                take = -(-len(bg) // (npairs - 1 - pi))
            else:
                take = len(bg)
            for _ in range(take):
                if bg:
                    bg.pop(0)()
            for k, j in enumerate((j0, j1)):
                nc.tensor.matmul(
                    psO[:, off:], vAug[:, j, :],
                    pt[:, k, off:],
                    start=(j == 0), stop=(j == nj - 1),
                )
        return psO, nxt

    def out(b, psO):
        # rows 64:128 of psO hold the softmax denominator, pre-broadcast.
        # 1/s as exp(-ln s) on ScalarE: same ACT table set as the softmax exp,
        # and ~3x faster than DVE reciprocal at this shape. Two column-halves
        # pipeline ACT -> DVE -> DMA and release the PV bank sooner.
        sl = slice(b * SBLK, (b + 1) * SBLK)
        lns = rpool.tile([D, SBLK], f32, tag="lns")
        nc.scalar.activation(lns[:], psO[D:P, :], AF.Ln)
        rcp = rpool.tile([D, SBLK], f32, tag="rcp")
        nc.scalar.activation(rcp[:], lns[:], AF.Exp, scale=-1.0)
        nc.vector.tensor_mul(yT[:, sl], psO[:D, :], rcp[:])
        if b == NSB - 1:
            # tail-critical store: halves on both HWDGE rings overlap receipts
            h0 = slice(b * SBLK, b * SBLK + SBLK // 2)
            h1 = slice(b * SBLK + SBLK // 2, (b + 1) * SBLK)
            nc.sync.dma_start(y[:, h0], yT[:, h0])
            nc.scalar.dma_start(y[:, h1], yT[:, h1])
        else:
            nc.sync.dma_start(y[:, sl], yT[:, sl])

    # Software pipeline: block b's attention interleaves block b+1's
    # projection matmuls between its exp-gated pairs, so the PE never
    # drains while ScalarE (the critical engine) streams exps. Block b-1's
    # output normalization rides as the first background step, placing its
    # ScalarE recip between this block's exps instead of at the boundary.
    # Block 0: emit only the Q/K projection up front; its V projection
    # rides as the first background steps of attn(0). The pair-0 PV only
    # consumes vAug tiles 0..1, which the bg pops emit before it.
    for step in qk_chunks(0):
        step()
    prev, nxt = attn(0, bg=v_chunks(0) + proj_chunks(1))
    for b in range(1, NSB):
        bg = [lambda p=prev, bb=b: out(bb - 1, p)]
        if b + 1 < NSB:
            bg += proj_chunks(b + 1)
        prev, nxt = attn(b, bg=bg, inflight=nxt)
    out(NSB - 1, prev)


def build_nc():
    from contextlib import ExitStack

    _patch_tile_drain()
    nc = bass.Bass(target_bir_lowering=False, enable_partition_id=False)
    xt = nc.dram_tensor("xt", [E, S], MMDT, kind="ExternalInput")
    wqk = nc.dram_tensor("wqk", [E, 2 * D], MMDT, kind="ExternalInput")
    wv = nc.dram_tensor("wv", [E, D], MMDT, kind="ExternalInput")
    y = nc.dram_tensor("y", [D, S], f32, kind="ExternalOutput")
    with tile.TileContext(nc) as tc:
        with ExitStack() as ctx:
            _attention(ctx, tc, xt, wqk, wv, y)
    return nc


def make_in_maps(x, Wq, Wk, Wv):
    wqk = np.ascontiguousarray(np.concatenate([Wq, Wk], axis=1), dtype=MMNP)
    wv_c = np.ascontiguousarray(Wv, dtype=MMNP)
    x = np.asarray(x)
    return [
        {
            "xt": np.ascontiguousarray(x[b].T.astype(MMNP)),
            "wqk": wqk,
            "wv": wv_c,
        }
        for b in range(B)
    ]


_NC = None


def kernel(x, Wq, Wk, Wv, _trace=False, _tmpdir=None):
    from concourse.bass_utils import run_bass_kernel_spmd

    global _NC
    if _NC is None:
        _NC = build_nc()
        _split_multiwaits(_NC)  # walrus-only legalization; breaks CoreSim
    in_maps = make_in_maps(x, Wq, Wk, Wv)
    res = run_bass_kernel_spmd(
        _NC, in_maps, core_ids=list(range(B)), trace=_trace, tmpdir=_tmpdir
    )
    out = np.ascontiguousarray(
        np.stack([r["y"].T for r in res.results], axis=0), dtype=np.float32
    )
    if _trace:
        kernel.last_results = res
    return out



# revision 33
# speedup vs baseline: 1.1941x; 1.1941x over previous
"""Single-head causal attention on 8 trn2 NeuronCores (one batch element per core).

Problem: x [8, 2048, 1024], Wq/Wk/Wv [1024, 64] -> out [8, 2048, 64]
  q = x@Wq; k = x@Wk; v = x@Wv; out = causal_softmax(q k^T / sqrt(64)) @ v

Strategy (per core, batch-parallel across the 8 cores):
  - Host pre-transposes each core's x to x^T [E, S] so the QKV projections can
    contract over E with E on SBUF partitions (no on-chip transpose of x).
  - Projections on the PE as fp32r matmuls: Q^T and K^T are produced
    *duplicated* across partition halves (lhsT = [Wq|Wq]) so the score
    matmuls can be packed two-at-a-time into disjoint PE row groups.
  - Scores are computed transposed (P^T[kv, q]) so that softmax normalization
    can ride the PV matmul: V is augmented with a ones column, making row 64
    of the PV output the softmax denominator. No max-subtraction is needed
    (scores are O(1) by construction; exp cannot overflow fp32).
  - exp on ScalarE straight out of PSUM with the 1/sqrt(D) scale folded in.
  - Causal masking is a multiplicative 0/1 mask applied after exp, only on
    diagonal tiles, sliced from one precomputed [128, 1024] step mask.
  - PV accumulates out^T in PSUM; a PE transpose brings it back to natural
    layout where the per-query normalizer lands on the partition dim, so the
    divide is a reciprocal + per-partition tensor_scalar multiply.
"""

import numpy as np

import concourse.bass as bass
import concourse.mybir as mybir
import concourse.tile as tile
from concourse.vector_clock import ScopedClock

S = 2048  # sequence length
E = 1024  # embed dim
D = 64    # head size
B = 8     # batch == number of cores
P = 128   # SBUF partitions
SBLK = 512         # q-block / s-block width (max fp32 matmul moving dim)
EC = E // P        # 8 contraction chunks
NSB = S // SBLK    # 4 s-blocks
NJT = S // P       # 16 kv tiles

f32 = mybir.dt.float32
f32r = mybir.dt.float32r
f16 = mybir.dt.float16
MMDT = f16          # dtype of all large-matmul operands
MMNP = np.float16   # matching numpy dtype for host-side prep
AF = mybir.ActivationFunctionType

_PATCHED = False


def _patch_tile_drain():
    """The walrus build in this container rejects instructions carrying more
    than one sem wait on the Tile exit Drain. Split the waits across a chain
    of drains, one wait each."""
    global _PATCHED
    if _PATCHED:
        return
    _PATCHED = True

    def _drain_and_barrier(self, tick_clock, wait_clock):
        drain_inst = self.nc.sync.drain()
        wait_clock.add_sem_waits(
            drain_inst.ins, ScopedClock({None: tick_clock.global_clock})
        )
        ins = drain_inst.ins
        si = ins.sync_info
        if si is not None and si.on_wait is not None and len(si.on_wait) > 1:
            waits = list(si.on_wait)
            ins.sync_info = mybir.SyncInfo(
                on_wait=[waits[0]], on_update=list(si.on_update or [])
            )
            for w in waits[1:]:
                d2 = self.nc.sync.drain()
                d2.ins.sync_info = mybir.SyncInfo(on_wait=[w], on_update=[])
        self.nc.all_engine_barrier()
        assert self.sems is not None
        popped = self.nc._tile_sem_poison_stack.pop()
        assert popped is self._sem_poison
        self.nc.clear_and_free_semaphores(list(self.sems.allocated().values()))
        self.nc.all_engine_barrier()

    tile.TileContext._drain_and_barrier = _drain_and_barrier


def _split_multiwaits(nc):
    """This container's walrus rejects instructions carrying more than one
    sem wait (setupSyncWait: 'Too many sync wait commands'). Hoist all but
    the last wait of every instruction onto same-engine NoOps placed
    immediately before it — the engine sequencer processes them in order,
    which is semantically identical."""
    ctr = 0
    for f in nc.m.functions:
        for bb in f.blocks:
            out = []
            changed = False
            for inst in bb.instructions:
                si = inst.sync_info
                if si is not None and si.on_wait is not None and len(si.on_wait) > 1:
                    waits = list(si.on_wait)
                    for w in waits[:-1]:
                        nop = mybir.InstNoOp(name=f"I-waitsplit-{ctr}")
                        ctr += 1
                        nop.engine = inst.engine
                        nop.sync_info = mybir.SyncInfo(on_wait=[w], on_update=[])
                        out.append(nop)
                    inst.sync_info = mybir.SyncInfo(
                        on_wait=[waits[-1]], on_update=list(si.on_update or [])
                    )
                    changed = True
                out.append(inst)
            if changed:
                bb.instructions = out


def _attention(ctx, tc, xt, wqk, wv, y):
    nc = tc.nc
    scale = 1.0 / np.sqrt(D)

    persist = ctx.enter_context(tc.tile_pool(name="persist", bufs=1))
    xpool = ctx.enter_context(tc.tile_pool(name="xts", bufs=1))
    ppool = ctx.enter_context(tc.tile_pool(name="pp", bufs=6))
    opool = ctx.enter_context(tc.tile_pool(name="ot", bufs=2))
    rpool = ctx.enter_context(tc.tile_pool(name="rec", bufs=8))
    # PSUM budget (8 banks): psproj 1 (QK/V serialize naturally through the
    # proj step order; the warmup target shares the slot) + psscore 2x2 +
    # pspv 1 + pstr 2 (double-buffered transpose targets so the
    # PE-transpose -> DVE-copy chain pipelines instead of round-tripping).
    psproj = ctx.enter_context(tc.tile_pool(name="psproj", bufs=1, space="PSUM"))
    psscore = ctx.enter_context(tc.tile_pool(name="psscore", bufs=2, space="PSUM"))
    pspv = ctx.enter_context(tc.tile_pool(name="pspv", bufs=1, space="PSUM"))
    pstr = ctx.enter_context(tc.tile_pool(name="pstr", bufs=2, space="PSUM"))

    # ---- weights (dual queue: wqk on sync, wv on scalar) ----------------
    wqk_sb = persist.tile([P, EC, 2 * D], MMDT, tag="wqk")  # [Wq|Wk] packed
    wv_sb = persist.tile([P, EC, D], MMDT, tag="wv")
    nc.sync.dma_start(wqk_sb[:], wqk.rearrange("(c p) m -> p c m", p=P))
    nc.scalar.dma_start(wv_sb[:], wv.rearrange("(c p) m -> p c m", p=P))

    # ---- PE warm-up: keep HAM busy while the input streams in -----------
    warm_in = persist.tile([P, SBLK], MMDT, tag="warm")
    nc.vector.memset(warm_in[:], 0.25)
    # dummy activation: pull the ~1.3us ACT table load into the DMA phase
    warm_act = rpool.tile([P, 8], f32, tag="warmact")
    nc.scalar.activation(warm_act[:], warm_in[:, :8], AF.Exp, scale=1.0)
    wt = psproj.tile([P, SBLK], f32, tag="proj")
    for _ in range(8):
        nc.tensor.matmul(wt[:], warm_in[:, :P], warm_in[:], start=True, stop=True)

    # ---- constants -------------------------------------------------------
    ident = persist.tile([P, P], f32, tag="ident")
    nc.gpsimd.memset(ident[:], 0.0)
    nc.gpsimd.affine_select(
        out=ident[:], in_=ident[:],
        compare_op=mybir.AluOpType.not_equal, fill=1.0,
        base=0, pattern=[[-1, P]], channel_multiplier=1,
    )
    ident16 = persist.tile([P, P], MMDT, tag="ident16")
    nc.vector.tensor_copy(ident16[:], ident[:])

    # causal step mask: maskW[jj, c] = 1 iff c >= jj + SBLK
    maskW = persist.tile([P, 2 * SBLK], f32, tag="maskw")
    nc.gpsimd.memset(maskW[:], 1.0)
    nc.gpsimd.affine_select(
        out=maskW[:], in_=maskW[:],
        compare_op=mybir.AluOpType.is_ge, fill=0.0,
        base=-SBLK, pattern=[[1, 2 * SBLK]], channel_multiplier=-1,
    )
    mask16 = persist.tile([P, 2 * SBLK], MMDT, tag="mask16")
    nc.vector.tensor_copy(mask16[:], maskW[:])

    # ---- persistent activations -----------------------------------------
    # qk: rows 0:64 = Q^T, rows 64:128 = K^T (straight from packed psum)
    qk = persist.tile([P, S], MMDT, tag="qk")
    # partition-shifted copies (SBUF->SBUF DMA): K^T at rows 0:64, Q^T at 64:128
    kTlo = persist.tile([D, S], MMDT, tag="ktlo")
    qThi = persist.tile([P, S], MMDT, tag="qthi")  # rows 64:128 used
    vT = persist.tile([D, S], MMDT, tag="vt")
    vAug = persist.tile([P, NJT, 2 * D], MMDT, tag="vaug")
    yT = persist.tile([D, S], f32, tag="ytout")
    ones_f32 = persist.tile([P, NJT, D], f32, tag="ones")
    nc.vector.memset(ones_f32[:], 1.0)
    nc.vector.tensor_copy(vAug[:, :, D:], ones_f32[:])

    # ---- stream x^T block-by-block -------------------------------------
    # Static DMAs land on separate logical queues and would all stream
    # CONCURRENTLY from t=0, sharing bandwidth fairly and starving block 0.
    # Gate block b's load on earlier compute via tiny WAW writes into the
    # destination tile: the DMA then fires only once the gate ran, keeping
    # HBM focused on the block the pipeline needs next.
    xt_r = xt.rearrange("(c p) s -> p c s", p=P)
    xts = []
    for b in range(NSB):
        xts_b = xpool.tile([P, EC, SBLK], MMDT, tag=f"xts{b}", name=f"xts{b}")
        xts.append(xts_b)

    def stream_x(b):
        sl_b = slice(b * SBLK, (b + 1) * SBLK)
        nc.sync.dma_start(xts[b][:, : EC // 2, :], xt_r[:, : EC // 2, sl_b])
        nc.scalar.dma_start(xts[b][:, EC // 2 :, :], xt_r[:, EC // 2 :, sl_b])

    def gate_x(b, src):
        nc.vector.tensor_copy(xts[b][0:1, 0, 0:1], src)
        nc.vector.tensor_copy(xts[b][0:1, EC // 2, 0:1], src)

    def stream_x_gated(b):
        # SWDGE (gpsimd-generated) DMA: unlike static-queue HWDGE DMAs,
        # which all fire at t=0 and share bandwidth fairly, the SWDGE
        # trigger honors its sem waits, so the gate really delays the
        # transfer and HBM stays focused on the block the pipeline needs.
        sl_b = slice(b * SBLK, (b + 1) * SBLK)
        nc.gpsimd.dma_start(xts[b][:, : EC // 2, :], xt_r[:, : EC // 2, sl_b])
        nc.gpsimd.dma_start(xts[b][:, EC // 2 :, :], xt_r[:, EC // 2 :, sl_b])

    stream_x(0)
    gate_x(1, wt[0:1, 0:1])  # x1 fires once the warmup matmuls finish
    stream_x_gated(1)

    def qk_chunks(b):
        """Emit-steps for s-block b's Q/K projection."""
        sl = slice(b * SBLK, (b + 1) * SBLK)
        steps = []
        psQK = psproj.tile([P, SBLK], f32, tag="proj")

        def qk_mms(e0, psQK=psQK):
            for e in (e0, e0 + 1):
                nc.tensor.matmul(
                    psQK[:], wqk_sb[:, e, :], xts[b][:, e, :],
                    start=(e == 0), stop=(e == EC - 1),
                )

        for e0 in range(0, EC, 2):
            steps.append(lambda e0=e0: qk_mms(e0))

        def qk_out():
            nc.vector.tensor_copy(qk[:, sl], psQK[:])

        # partition-shifted copies on DVE (APs with equal partition counts
        # may live at different base partitions); f16 source for 2x rate
        def qk_shift_k():
            nc.vector.tensor_copy(kTlo[:, sl], qk[D:P, sl])

        def qk_shift_q():
            nc.vector.tensor_copy(qThi[D:P, sl], qk[:D, sl])

        steps.append(qk_out)
        steps.append(qk_shift_k)
        steps.append(qk_shift_q)
        if 1 <= b < NSB - 1:
            def gate_next():
                # release block b+1's x stream once this block's projection
                # is through the PE
                gate_x(b + 1, psQK[0:1, 0:1])
                stream_x_gated(b + 1)

            steps.append(gate_next)
        return steps

    def v_chunks(b):
        """Emit-steps for s-block b's V projection."""
        sl = slice(b * SBLK, (b + 1) * SBLK)
        steps = []
        psV = psproj.tile([P, SBLK], f32, tag="proj")

        def v_mms(e0, psV=psV):
            for e in (e0, e0 + 1):
                nc.tensor.matmul(
                    psV[:D, :], wv_sb[:, e, :], xts[b][:, e, :],
                    start=(e == 0), stop=(e == EC - 1),
                )

        for e0 in range(0, EC, 2):
            steps.append(lambda e0=e0: v_mms(e0))

        def v_out():
            nc.vector.tensor_copy(vT[:, sl], psV[:D, :])

        steps.append(v_out)

        def v_tr(t):
            j = 4 * b + t
            psv_t = pstr.tile([P, D], MMDT, tag="tr")
            nc.tensor.transpose(
                psv_t[:], vT[:, j * P : (j + 1) * P], ident16[:D, :D]
            )
            nc.vector.tensor_copy(vAug[:, j, :D], psv_t[:])

        for t in range(4):
            steps.append(lambda t=t: v_tr(t))
        return steps

    def proj_chunks(b):
        return qk_chunks(b) + v_chunks(b)

    def scores_pair_for(b, pi):
        """Scores for kv tiles (2pi, 2pi+1) against q-block b. The second
        diagonal pair (tiles t=2,3) narrows BOTH members to cols >= 2P so
        its exp and mask each collapse to one strided instruction; all other
        pairs run full width. Narrowed matmuls still cover the full exp-read
        region, so exp never reads stale psum."""
        j0, j1 = 2 * pi, 2 * pi + 1
        o0 = 2 * P if (j0 - 4 * b) == 2 else 0
        o1 = 3 * P if (j1 - 4 * b) == 3 else 0
        ps = psscore.tile([P, 2, SBLK], f32, tag="score")
        q0 = slice(b * SBLK + o0, (b + 1) * SBLK)
        q1 = slice(b * SBLK + o1, (b + 1) * SBLK)
        # two PE row-groups: rows 0:64 (kTlo/qk) and 64:128 (qk/qThi)
        nc.tensor.matmul(
            ps[:, 0, o0:], kTlo[:, j0 * P : (j0 + 1) * P], qk[:D, q0],
        )
        nc.tensor.matmul(
            ps[:, 1, o1:], qk[D:P, j1 * P : (j1 + 1) * P], qThi[D:P, q1],
        )
        return (j0, j1, ps, (o0, o1))

    def attn(b, bg=(), inflight=None):
        bg = list(bg)
        nj = 4 * b + 4
        psO = pspv.tile([P, SBLK], f32, tag="pv")
        npairs = nj // 2
        if inflight is None:
            inflight = scores_pair_for(b, 0)
        nxt = None
        for pi in range(npairs):
            j0, j1, ps, offs = inflight
            pt = ppool.tile([P, 2, SBLK], MMDT, tag="pt")
            if offs == (0, 0):
                # fully-computed pair: one contiguous exp over both banks
                nc.scalar.activation(pt[:], ps[:], AF.Exp, scale=float(scale))
            else:
                # diagonal t2/t3 pair: exp only the causally-reachable columns
                for k, off in enumerate(offs):
                    nc.scalar.activation(
                        pt[:, k, off:], ps[:, k, off:], AF.Exp,
                        scale=float(scale),
                    )
            for k, j in enumerate((j0, j1)):
                t = j - 4 * b
                if t >= 0:
                    off = offs[k]
                    nc.vector.tensor_mul(
                        pt[:, k, off:],
                        pt[:, k, off:],
                        mask16[:, SBLK - t * P + off : 2 * SBLK - t * P],
                    )
            if pi + 1 < npairs:
                inflight = scores_pair_for(b, pi + 1)
            elif b + 1 < NSB:
                # cross-block prefetch: next block's first score pair goes
                # out before this block's last PV, so ScalarE rolls into the
                # next block's exps with no boundary bubble. Requires bg
                # (incl. next block's qk_out/shifts) fully drained by now.
                nxt = scores_pair_for(b + 1, 0)
            # independent next-block projection work keeps the PE busy
            # while this pair's exp runs on ScalarE; drain everything
            # before the last pair so the prefetched scores see their data
            if pi < npairs - 1:
                take = -(-len(bg) // (npairs - 1 - pi))
            else:
                take = len(bg)
            for _ in range(take):
                if bg:
                    bg.pop(0)()
            for k, j in enumerate((j0, j1)):
                off = offs[k]
                nc.tensor.matmul(
                    psO[:, off:], vAug[:, j, :],
                    pt[:, k, off:],
                    start=(j == 0), stop=(j == nj - 1),
                )
        return psO, nxt

    def out(b, psO):
        # rows 64:128 of psO hold the softmax denominator, pre-broadcast.
        # 1/s as exp(-ln s) on ScalarE: same ACT table set as the softmax exp,
        # and ~3x faster than DVE reciprocal at this shape. Two column-halves
        # pipeline ACT -> DVE -> DMA and release the PV bank sooner.
        sl = slice(b * SBLK, (b + 1) * SBLK)
        lns = rpool.tile([D, SBLK], f32, tag="lns")
        nc.scalar.activation(lns[:], psO[D:P, :], AF.Ln)
        rcp = rpool.tile([D, SBLK], f32, tag="rcp")
        nc.scalar.activation(rcp[:], lns[:], AF.Exp, scale=-1.0)
        nc.vector.tensor_mul(yT[:, sl], psO[:D, :], rcp[:])
        if b == NSB - 1:
            # tail-critical store: halves on both HWDGE rings overlap receipts
            h0 = slice(b * SBLK, b * SBLK + SBLK // 2)
            h1 = slice(b * SBLK + SBLK // 2, (b + 1) * SBLK)
            nc.sync.dma_start(y[:, h0], yT[:, h0])
            nc.scalar.dma_start(y[:, h1], yT[:, h1])
        else:
            nc.sync.dma_start(y[:, sl], yT[:, sl])

    # Software pipeline: block b's attention interleaves block b+1's
    # projection matmuls between its exp-gated pairs, so the PE never
    # drains while ScalarE (the critical engine) streams exps. Block b-1's
    # output normalization rides as the first background step, placing its
    # ScalarE recip between this block's exps instead of at the boundary.
    # Block 0: emit only the Q/K projection up front; its V projection
    # rides as the first background steps of attn(0). The pair-0 PV only
    # consumes vAug tiles 0..1, which the bg pops emit before it.
    for step in qk_chunks(0):
        step()
    prev, nxt = attn(0, bg=v_chunks(0) + proj_chunks(1))
    for b in range(1, NSB):
        bg = [lambda p=prev, bb=b: out(bb - 1, p)]
        if b + 1 < NSB:
            bg += proj_chunks(b + 1)
        prev, nxt = attn(b, bg=bg, inflight=nxt)
    out(NSB - 1, prev)


def build_nc():
    from contextlib import ExitStack

    _patch_tile_drain()
    nc = bass.Bass(target_bir_lowering=False, enable_partition_id=False)
    xt = nc.dram_tensor("xt", [E, S], MMDT, kind="ExternalInput")
    wqk = nc.dram_tensor("wqk", [E, 2 * D], MMDT, kind="ExternalInput")
    wv = nc.dram_tensor("wv", [E, D], MMDT, kind="ExternalInput")
    y = nc.dram_tensor("y", [D, S], f32, kind="ExternalOutput")
    with tile.TileContext(nc) as tc:
        with ExitStack() as ctx:
            _attention(ctx, tc, xt, wqk, wv, y)
    return nc


def make_in_maps(x, Wq, Wk, Wv):
    wqk = np.ascontiguousarray(np.concatenate([Wq, Wk], axis=1), dtype=MMNP)
    wv_c = np.ascontiguousarray(Wv, dtype=MMNP)
    x = np.asarray(x)
    return [
        {
            "xt": np.ascontiguousarray(x[b].T.astype(MMNP)),
            "wqk": wqk,
            "wv": wv_c,
        }
        for b in range(B)
    ]


_NC = None


def kernel(x, Wq, Wk, Wv, _trace=False, _tmpdir=None):
    from concourse.bass_utils import run_bass_kernel_spmd

    global _NC
    if _NC is None:
        _NC = build_nc()
        _split_multiwaits(_NC)  # walrus-only legalization; breaks CoreSim
    in_maps = make_in_maps(x, Wq, Wk, Wv)
    res = run_bass_kernel_spmd(
        _NC, in_maps, core_ids=list(range(B)), trace=_trace, tmpdir=_tmpdir
    )
    out = np.ascontiguousarray(
        np.stack([r["y"].T for r in res.results], axis=0), dtype=np.float32
    )
    if _trace:
        kernel.last_results = res
    return out



# revision 36
# speedup vs baseline: 1.2129x; 1.0157x over previous
"""Single-head causal attention on 8 trn2 NeuronCores (one batch element per core).

Problem: x [8, 2048, 1024], Wq/Wk/Wv [1024, 64] -> out [8, 2048, 64]
  q = x@Wq; k = x@Wk; v = x@Wv; out = causal_softmax(q k^T / sqrt(64)) @ v

Strategy (per core, batch-parallel across the 8 cores):
  - Host pre-transposes each core's x to x^T [E, S] so the QKV projections can
    contract over E with E on SBUF partitions (no on-chip transpose of x).
  - Projections on the PE as fp32r matmuls: Q^T and K^T are produced
    *duplicated* across partition halves (lhsT = [Wq|Wq]) so the score
    matmuls can be packed two-at-a-time into disjoint PE row groups.
  - Scores are computed transposed (P^T[kv, q]) so that softmax normalization
    can ride the PV matmul: V is augmented with a ones column, making row 64
    of the PV output the softmax denominator. No max-subtraction is needed
    (scores are O(1) by construction; exp cannot overflow fp32).
  - exp on ScalarE straight out of PSUM with the 1/sqrt(D) scale folded in.
  - Causal masking is a multiplicative 0/1 mask applied after exp, only on
    diagonal tiles, sliced from one precomputed [128, 1024] step mask.
  - PV accumulates out^T in PSUM; a PE transpose brings it back to natural
    layout where the per-query normalizer lands on the partition dim, so the
    divide is a reciprocal + per-partition tensor_scalar multiply.
"""

import numpy as np

import concourse.bass as bass
import concourse.mybir as mybir
import concourse.tile as tile
from concourse.vector_clock import ScopedClock

S = 2048  # sequence length
E = 1024  # embed dim
D = 64    # head size
B = 8     # batch == number of cores
P = 128   # SBUF partitions
SBLK = 512         # q-block / s-block width (max fp32 matmul moving dim)
EC = E // P        # 8 contraction chunks
NSB = S // SBLK    # 4 s-blocks
NJT = S // P       # 16 kv tiles

f32 = mybir.dt.float32
f32r = mybir.dt.float32r
f16 = mybir.dt.float16
MMDT = f16          # dtype of all large-matmul operands
MMNP = np.float16   # matching numpy dtype for host-side prep
AF = mybir.ActivationFunctionType

_PATCHED = False


def _patch_tile_drain():
    """The walrus build in this container rejects instructions carrying more
    than one sem wait on the Tile exit Drain. Split the waits across a chain
    of drains, one wait each."""
    global _PATCHED
    if _PATCHED:
        return
    _PATCHED = True

    def _drain_and_barrier(self, tick_clock, wait_clock):
        drain_inst = self.nc.sync.drain()
        wait_clock.add_sem_waits(
            drain_inst.ins, ScopedClock({None: tick_clock.global_clock})
        )
        ins = drain_inst.ins
        si = ins.sync_info
        if si is not None and si.on_wait is not None and len(si.on_wait) > 1:
            waits = list(si.on_wait)
            ins.sync_info = mybir.SyncInfo(
                on_wait=[waits[0]], on_update=list(si.on_update or [])
            )
            for w in waits[1:]:
                d2 = self.nc.sync.drain()
                d2.ins.sync_info = mybir.SyncInfo(on_wait=[w], on_update=[])
        self.nc.all_engine_barrier()
        assert self.sems is not None
        popped = self.nc._tile_sem_poison_stack.pop()
        assert popped is self._sem_poison
        self.nc.clear_and_free_semaphores(list(self.sems.allocated().values()))
        self.nc.all_engine_barrier()

    tile.TileContext._drain_and_barrier = _drain_and_barrier


def _split_multiwaits(nc):
    """This container's walrus rejects instructions carrying more than one
    sem wait (setupSyncWait: 'Too many sync wait commands'). Hoist all but
    the last wait of every instruction onto same-engine NoOps placed
    immediately before it — the engine sequencer processes them in order,
    which is semantically identical."""
    ctr = 0
    for f in nc.m.functions:
        for bb in f.blocks:
            out = []
            changed = False
            for inst in bb.instructions:
                si = inst.sync_info
                if si is not None and si.on_wait is not None and len(si.on_wait) > 1:
                    waits = list(si.on_wait)
                    for w in waits[:-1]:
                        nop = mybir.InstNoOp(name=f"I-waitsplit-{ctr}")
                        ctr += 1
                        nop.engine = inst.engine
                        nop.sync_info = mybir.SyncInfo(on_wait=[w], on_update=[])
                        out.append(nop)
                    inst.sync_info = mybir.SyncInfo(
                        on_wait=[waits[-1]], on_update=list(si.on_update or [])
                    )
                    changed = True
                out.append(inst)
            if changed:
                bb.instructions = out


def _attention(ctx, tc, xt, wqk, wv, y):
    nc = tc.nc
    scale = 1.0 / np.sqrt(D)

    persist = ctx.enter_context(tc.tile_pool(name="persist", bufs=1))
    xpool = ctx.enter_context(tc.tile_pool(name="xts", bufs=1))
    ppool = ctx.enter_context(tc.tile_pool(name="pp", bufs=6))
    opool = ctx.enter_context(tc.tile_pool(name="ot", bufs=2))
    rpool = ctx.enter_context(tc.tile_pool(name="rec", bufs=8))
    # PSUM budget (8 banks): psproj 2 (separate QK/V slots, so the V matmuls
    # never wait on the DVE cast/shift readers of the QK slot) + psscore 2x2
    # + pspv 1 + pstr 2x128B (double-buffered transpose targets so the
    # PE-transpose -> DVE-copy chain pipelines instead of round-tripping).
    psproj = ctx.enter_context(tc.tile_pool(name="psproj", bufs=2, space="PSUM"))
    psscore = ctx.enter_context(tc.tile_pool(name="psscore", bufs=2, space="PSUM"))
    pspv = ctx.enter_context(tc.tile_pool(name="pspv", bufs=1, space="PSUM"))
    pstr = ctx.enter_context(tc.tile_pool(name="pstr", bufs=1, space="PSUM"))

    # ---- weights (dual queue: wqk on sync, wv on scalar) ----------------
    wqk_sb = persist.tile([P, EC, 2 * D], MMDT, tag="wqk")  # [Wq|Wk] packed
    wv_sb = persist.tile([P, EC, D], MMDT, tag="wv")
    nc.sync.dma_start(wqk_sb[:], wqk.rearrange("(c p) m -> p c m", p=P))
    nc.scalar.dma_start(wv_sb[:], wv.rearrange("(c p) m -> p c m", p=P))

    # ---- PE warm-up: keep HAM busy while the input streams in -----------
    warm_in = persist.tile([P, SBLK], MMDT, tag="warm")
    nc.vector.memset(warm_in[:], 0.25)
    # dummy activation: pull the ~1.3us ACT table load into the DMA phase
    warm_act = rpool.tile([P, 8], f32, tag="warmact")
    nc.scalar.activation(warm_act[:], warm_in[:, :8], AF.Exp, scale=1.0)
    wt = psproj.tile([P, SBLK], f32, tag="proj")
    for _ in range(8):
        nc.tensor.matmul(wt[:], warm_in[:, :P], warm_in[:], start=True, stop=True)

    # ---- constants -------------------------------------------------------
    ident = persist.tile([P, P], f32, tag="ident")
    nc.gpsimd.memset(ident[:], 0.0)
    nc.gpsimd.affine_select(
        out=ident[:], in_=ident[:],
        compare_op=mybir.AluOpType.not_equal, fill=1.0,
        base=0, pattern=[[-1, P]], channel_multiplier=1,
    )
    ident16 = persist.tile([P, P], MMDT, tag="ident16")
    nc.vector.tensor_copy(ident16[:], ident[:])

    # causal step mask: maskW[jj, c] = 1 iff c >= jj + SBLK
    maskW = persist.tile([P, 2 * SBLK], f32, tag="maskw")
    nc.gpsimd.memset(maskW[:], 1.0)
    nc.gpsimd.affine_select(
        out=maskW[:], in_=maskW[:],
        compare_op=mybir.AluOpType.is_ge, fill=0.0,
        base=-SBLK, pattern=[[1, 2 * SBLK]], channel_multiplier=-1,
    )
    mask16 = persist.tile([P, 2 * SBLK], MMDT, tag="mask16")
    nc.vector.tensor_copy(mask16[:], maskW[:])

    # ---- persistent activations -----------------------------------------
    # qk: rows 0:64 = Q^T, rows 64:128 = K^T (straight from packed psum)
    qk = persist.tile([P, S], MMDT, tag="qk")
    # partition-shifted copies (SBUF->SBUF DMA): K^T at rows 0:64, Q^T at 64:128
    kTlo = persist.tile([D, S], MMDT, tag="ktlo")
    qThi = persist.tile([P, S], MMDT, tag="qthi")  # rows 64:128 used
    vT = persist.tile([D, S], MMDT, tag="vt")
    vAug = persist.tile([P, NJT, 2 * D], MMDT, tag="vaug")
    yT = persist.tile([D, S], f32, tag="ytout")
    ones_f32 = persist.tile([P, NJT, D], f32, tag="ones")
    nc.vector.memset(ones_f32[:], 1.0)
    nc.vector.tensor_copy(vAug[:, :, D:], ones_f32[:])

    # ---- stream x^T block-by-block -------------------------------------
    # Static DMAs land on separate logical queues and would all stream
    # CONCURRENTLY from t=0, sharing bandwidth fairly and starving block 0.
    # Gate block b's load on earlier compute via tiny WAW writes into the
    # destination tile: the DMA then fires only once the gate ran, keeping
    # HBM focused on the block the pipeline needs next.
    xt_r = xt.rearrange("(c p) s -> p c s", p=P)
    xts = []
    for b in range(NSB):
        xts_b = xpool.tile([P, EC, SBLK], MMDT, tag=f"xts{b}", name=f"xts{b}")
        xts.append(xts_b)

    def stream_x(b):
        sl_b = slice(b * SBLK, (b + 1) * SBLK)
        nc.sync.dma_start(xts[b][:, : EC // 2, :], xt_r[:, : EC // 2, sl_b])
        nc.scalar.dma_start(xts[b][:, EC // 2 :, :], xt_r[:, EC // 2 :, sl_b])

    def gate_x(b, src):
        nc.vector.tensor_copy(xts[b][0:1, 0, 0:1], src)
        nc.vector.tensor_copy(xts[b][0:1, EC // 2, 0:1], src)

    def stream_x_gated(b):
        # SWDGE (gpsimd-generated) DMA: unlike static-queue HWDGE DMAs,
        # which all fire at t=0 and share bandwidth fairly, the SWDGE
        # trigger honors its sem waits, so the gate really delays the
        # transfer and HBM stays focused on the block the pipeline needs.
        sl_b = slice(b * SBLK, (b + 1) * SBLK)
        nc.gpsimd.dma_start(xts[b][:, : EC // 2, :], xt_r[:, : EC // 2, sl_b])
        nc.gpsimd.dma_start(xts[b][:, EC // 2 :, :], xt_r[:, EC // 2 :, sl_b])

    stream_x(0)
    gate_x(1, wt[0:1, 0:1])  # x1 fires once the warmup matmuls finish
    stream_x_gated(1)

    def qk_chunks(b):
        """Emit-steps for s-block b's Q/K projection."""
        sl = slice(b * SBLK, (b + 1) * SBLK)
        steps = []
        psQK = psproj.tile([P, SBLK], f32, tag="proj")

        def qk_mms(e0, psQK=psQK):
            for e in (e0, e0 + 1):
                nc.tensor.matmul(
                    psQK[:], wqk_sb[:, e, :], xts[b][:, e, :],
                    start=(e == 0), stop=(e == EC - 1),
                )

        for e0 in range(0, EC, 2):
            steps.append(lambda e0=e0: qk_mms(e0))

        def qk_out():
            nc.vector.tensor_copy(qk[:, sl], psQK[:])

        # partition-shifted copies on DVE (APs with equal partition counts
        # may live at different base partitions); f16 source for 2x rate
        def qk_shift_k():
            nc.vector.tensor_copy(kTlo[:, sl], qk[D:P, sl])

        def qk_shift_q():
            nc.vector.tensor_copy(qThi[D:P, sl], qk[:D, sl])

        steps.append(qk_out)
        steps.append(qk_shift_k)
        steps.append(qk_shift_q)
        if 1 <= b < NSB - 1:
            def gate_next():
                # release block b+1's x stream once this block's projection
                # is through the PE
                gate_x(b + 1, psQK[0:1, 0:1])
                stream_x_gated(b + 1)

            steps.append(gate_next)
        return steps

    def v_chunks(b):
        """Emit-steps for s-block b's V projection."""
        sl = slice(b * SBLK, (b + 1) * SBLK)
        steps = []
        psV = psproj.tile([P, SBLK], f32, tag="proj")

        def v_mms(e0, psV=psV):
            for e in (e0, e0 + 1):
                nc.tensor.matmul(
                    psV[:D, :], wv_sb[:, e, :], xts[b][:, e, :],
                    start=(e == 0), stop=(e == EC - 1),
                )

        for e0 in range(0, EC, 2):
            steps.append(lambda e0=e0: v_mms(e0))

        def v_out():
            nc.vector.tensor_copy(vT[:, sl], psV[:D, :])

        steps.append(v_out)

        # one two-half transpose target per block: alternating halves give
        # double-buffered PE-transpose -> DVE-copy pipelining in one bank
        psv2 = pstr.tile([P, 2, D], MMDT, tag="tr")

        def v_tr(t):
            j = 4 * b + t
            nc.tensor.transpose(
                psv2[:, t % 2, :], vT[:, j * P : (j + 1) * P], ident16[:D, :D]
            )
            nc.vector.tensor_copy(vAug[:, j, :D], psv2[:, t % 2, :])

        for t in range(4):
            steps.append(lambda t=t: v_tr(t))
        return steps

    def proj_chunks(b):
        return qk_chunks(b) + v_chunks(b)

    def scores_pair_for(b, pi):
        """Scores for kv tiles (2pi, 2pi+1) against q-block b. The second
        diagonal pair (tiles t=2,3) narrows BOTH members to cols >= 2P so
        its exp and mask each collapse to one strided instruction; all other
        pairs run full width. Narrowed matmuls still cover the full exp-read
        region, so exp never reads stale psum."""
        j0, j1 = 2 * pi, 2 * pi + 1
        o0 = 2 * P if (j0 - 4 * b) == 2 else 0
        o1 = 3 * P if (j1 - 4 * b) == 3 else 0
        ps = psscore.tile([P, 2, SBLK], f32, tag="score")
        q0 = slice(b * SBLK + o0, (b + 1) * SBLK)
        q1 = slice(b * SBLK + o1, (b + 1) * SBLK)
        # two PE row-groups: rows 0:64 (kTlo/qk) and 64:128 (qk/qThi)
        nc.tensor.matmul(
            ps[:, 0, o0:], kTlo[:, j0 * P : (j0 + 1) * P], qk[:D, q0],
        )
        nc.tensor.matmul(
            ps[:, 1, o1:], qk[D:P, j1 * P : (j1 + 1) * P], qThi[D:P, q1],
        )
        return (j0, j1, ps, (o0, o1))

    def attn(b, bg=(), inflight=None):
        bg = list(bg)
        nj = 4 * b + 4
        psO = pspv.tile([P, SBLK], f32, tag="pv")
        npairs = nj // 2
        if inflight is None:
            inflight = scores_pair_for(b, 0)
        nxt = None
        for pi in range(npairs):
            j0, j1, ps, offs = inflight
            pt = ppool.tile([P, 2, SBLK], MMDT, tag="pt")
            if offs == (0, 0):
                # fully-computed pair: one contiguous exp over both banks
                nc.scalar.activation(pt[:], ps[:], AF.Exp, scale=float(scale))
            else:
                # diagonal t2/t3 pair: exp only the causally-reachable columns
                for k, off in enumerate(offs):
                    nc.scalar.activation(
                        pt[:, k, off:], ps[:, k, off:], AF.Exp,
                        scale=float(scale),
                    )
            for k, j in enumerate((j0, j1)):
                t = j - 4 * b
                if t >= 0:
                    off = offs[k]
                    nc.vector.tensor_mul(
                        pt[:, k, off:],
                        pt[:, k, off:],
                        mask16[:, SBLK - t * P + off : 2 * SBLK - t * P],
                    )
            if pi + 1 < npairs:
                inflight = scores_pair_for(b, pi + 1)
            elif b + 1 < NSB:
                # cross-block prefetch: next block's first score pair goes
                # out before this block's last PV, so ScalarE rolls into the
                # next block's exps with no boundary bubble. Requires bg
                # (incl. next block's qk_out/shifts) fully drained by now.
                nxt = scores_pair_for(b + 1, 0)
            # independent next-block projection work keeps the PE busy
            # while this pair's exp runs on ScalarE; drain everything
            # before the last pair so the prefetched scores see their data
            if pi < npairs - 1:
                take = -(-len(bg) // (npairs - 1 - pi))
            else:
                take = len(bg)
            for _ in range(take):
                if bg:
                    bg.pop(0)()
            for k, j in enumerate((j0, j1)):
                off = offs[k]
                nc.tensor.matmul(
                    psO[:, off:], vAug[:, j, :],
                    pt[:, k, off:],
                    start=(j == 0), stop=(j == nj - 1),
                )
        return psO, nxt

    def out(b, psO):
        # rows 64:128 of psO hold the softmax denominator, pre-broadcast.
        # 1/s as exp(-ln s) on ScalarE: same ACT table set as the softmax exp,
        # and ~3x faster than DVE reciprocal at this shape. Two column-halves
        # pipeline ACT -> DVE -> DMA and release the PV bank sooner.
        sl = slice(b * SBLK, (b + 1) * SBLK)
        lns = rpool.tile([D, SBLK], f32, tag="lns")
        nc.scalar.activation(lns[:], psO[D:P, :], AF.Ln)
        rcp = rpool.tile([D, SBLK], f32, tag="rcp")
        nc.scalar.activation(rcp[:], lns[:], AF.Exp, scale=-1.0)
        nc.vector.tensor_mul(yT[:, sl], psO[:D, :], rcp[:])
        if b == NSB - 1:
            # tail-critical store: halves on both HWDGE rings overlap receipts
            h0 = slice(b * SBLK, b * SBLK + SBLK // 2)
            h1 = slice(b * SBLK + SBLK // 2, (b + 1) * SBLK)
            nc.sync.dma_start(y[:, h0], yT[:, h0])
            nc.scalar.dma_start(y[:, h1], yT[:, h1])
        else:
            nc.sync.dma_start(y[:, sl], yT[:, sl])

    # Software pipeline: block b's attention interleaves block b+1's
    # projection matmuls between its exp-gated pairs, so the PE never
    # drains while ScalarE (the critical engine) streams exps. Block b-1's
    # output normalization rides as the first background step, placing its
    # ScalarE recip between this block's exps instead of at the boundary.
    # Block 0: emit only the Q/K projection up front; its V projection
    # rides as the first background steps of attn(0). The pair-0 PV only
    # consumes vAug tiles 0..1, which the bg pops emit before it.
    for step in qk_chunks(0):
        step()
    prev, nxt = attn(0, bg=v_chunks(0) + proj_chunks(1))
    for b in range(1, NSB):
        bg = [lambda p=prev, bb=b: out(bb - 1, p)]
        if b + 1 < NSB:
            bg += proj_chunks(b + 1)
        prev, nxt = attn(b, bg=bg, inflight=nxt)
    out(NSB - 1, prev)


def build_nc():
    from contextlib import ExitStack

    _patch_tile_drain()
    nc = bass.Bass(target_bir_lowering=False, enable_partition_id=False)
    xt = nc.dram_tensor("xt", [E, S], MMDT, kind="ExternalInput")
    wqk = nc.dram_tensor("wqk", [E, 2 * D], MMDT, kind="ExternalInput")
    wv = nc.dram_tensor("wv", [E, D], MMDT, kind="ExternalInput")
    y = nc.dram_tensor("y", [D, S], f32, kind="ExternalOutput")
    with tile.TileContext(nc) as tc:
        with ExitStack() as ctx:
            _attention(ctx, tc, xt, wqk, wv, y)
    return nc


def make_in_maps(x, Wq, Wk, Wv):
    wqk = np.ascontiguousarray(np.concatenate([Wq, Wk], axis=1), dtype=MMNP)
    wv_c = np.ascontiguousarray(Wv, dtype=MMNP)
    x = np.asarray(x)
    return [
        {
            "xt": np.ascontiguousarray(x[b].T.astype(MMNP)),
            "wqk": wqk,
            "wv": wv_c,
        }
        for b in range(B)
    ]


_NC = None


def kernel(x, Wq, Wk, Wv, _trace=False, _tmpdir=None):
    from concourse.bass_utils import run_bass_kernel_spmd

    global _NC
    if _NC is None:
        _NC = build_nc()
        _split_multiwaits(_NC)  # walrus-only legalization; breaks CoreSim
    in_maps = make_in_maps(x, Wq, Wk, Wv)
    res = run_bass_kernel_spmd(
        _NC, in_maps, core_ids=list(range(B)), trace=_trace, tmpdir=_tmpdir
    )
    out = np.ascontiguousarray(
        np.stack([r["y"].T for r in res.results], axis=0), dtype=np.float32
    )
    if _trace:
        kernel.last_results = res
    return out



# revision 37
# speedup vs baseline: 1.2161x; 1.0027x over previous
"""Single-head causal attention on 8 trn2 NeuronCores (one batch element per core).

Problem: x [8, 2048, 1024], Wq/Wk/Wv [1024, 64] -> out [8, 2048, 64]
  q = x@Wq; k = x@Wk; v = x@Wv; out = causal_softmax(q k^T / sqrt(64)) @ v

Strategy (per core, batch-parallel across the 8 cores):
  - Host pre-transposes each core's x to x^T [E, S] so the QKV projections can
    contract over E with E on SBUF partitions (no on-chip transpose of x).
  - Projections on the PE as fp32r matmuls: Q^T and K^T are produced
    *duplicated* across partition halves (lhsT = [Wq|Wq]) so the score
    matmuls can be packed two-at-a-time into disjoint PE row groups.
  - Scores are computed transposed (P^T[kv, q]) so that softmax normalization
    can ride the PV matmul: V is augmented with a ones column, making row 64
    of the PV output the softmax denominator. No max-subtraction is needed
    (scores are O(1) by construction; exp cannot overflow fp32).
  - exp on ScalarE straight out of PSUM with the 1/sqrt(D) scale folded in.
  - Causal masking is a multiplicative 0/1 mask applied after exp, only on
    diagonal tiles, sliced from one precomputed [128, 1024] step mask.
  - PV accumulates out^T in PSUM; a PE transpose brings it back to natural
    layout where the per-query normalizer lands on the partition dim, so the
    divide is a reciprocal + per-partition tensor_scalar multiply.
"""

import numpy as np

import concourse.bass as bass
import concourse.mybir as mybir
import concourse.tile as tile
from concourse.vector_clock import ScopedClock

S = 2048  # sequence length
E = 1024  # embed dim
D = 64    # head size
B = 8     # batch == number of cores
P = 128   # SBUF partitions
SBLK = 512         # q-block / s-block width (max fp32 matmul moving dim)
EC = E // P        # 8 contraction chunks
NSB = S // SBLK    # 4 s-blocks
NJT = S // P       # 16 kv tiles

f32 = mybir.dt.float32
f32r = mybir.dt.float32r
f16 = mybir.dt.float16
MMDT = f16          # dtype of all large-matmul operands
MMNP = np.float16   # matching numpy dtype for host-side prep
AF = mybir.ActivationFunctionType

_PATCHED = False


def _patch_tile_drain():
    """The walrus build in this container rejects instructions carrying more
    than one sem wait on the Tile exit Drain. Split the waits across a chain
    of drains, one wait each."""
    global _PATCHED
    if _PATCHED:
        return
    _PATCHED = True

    def _drain_and_barrier(self, tick_clock, wait_clock):
        drain_inst = self.nc.sync.drain()
        wait_clock.add_sem_waits(
            drain_inst.ins, ScopedClock({None: tick_clock.global_clock})
        )
        ins = drain_inst.ins
        si = ins.sync_info
        if si is not None and si.on_wait is not None and len(si.on_wait) > 1:
            waits = list(si.on_wait)
            ins.sync_info = mybir.SyncInfo(
                on_wait=[waits[0]], on_update=list(si.on_update or [])
            )
            for w in waits[1:]:
                d2 = self.nc.sync.drain()
                d2.ins.sync_info = mybir.SyncInfo(on_wait=[w], on_update=[])
        self.nc.all_engine_barrier()
        assert self.sems is not None
        popped = self.nc._tile_sem_poison_stack.pop()
        assert popped is self._sem_poison
        self.nc.clear_and_free_semaphores(list(self.sems.allocated().values()))
        self.nc.all_engine_barrier()

    tile.TileContext._drain_and_barrier = _drain_and_barrier


def _split_multiwaits(nc):
    """This container's walrus rejects instructions carrying more than one
    sem wait (setupSyncWait: 'Too many sync wait commands'). Hoist all but
    the last wait of every instruction onto same-engine NoOps placed
    immediately before it — the engine sequencer processes them in order,
    which is semantically identical."""
    ctr = 0
    for f in nc.m.functions:
        for bb in f.blocks:
            out = []
            changed = False
            for inst in bb.instructions:
                si = inst.sync_info
                if si is not None and si.on_wait is not None and len(si.on_wait) > 1:
                    waits = list(si.on_wait)
                    for w in waits[:-1]:
                        nop = mybir.InstNoOp(name=f"I-waitsplit-{ctr}")
                        ctr += 1
                        nop.engine = inst.engine
                        nop.sync_info = mybir.SyncInfo(on_wait=[w], on_update=[])
                        out.append(nop)
                    inst.sync_info = mybir.SyncInfo(
                        on_wait=[waits[-1]], on_update=list(si.on_update or [])
                    )
                    changed = True
                out.append(inst)
            if changed:
                bb.instructions = out


def _attention(ctx, tc, xt, wqk, wv, y):
    nc = tc.nc
    scale = 1.0 / np.sqrt(D)

    persist = ctx.enter_context(tc.tile_pool(name="persist", bufs=1))
    xpool = ctx.enter_context(tc.tile_pool(name="xts", bufs=1))
    ppool = ctx.enter_context(tc.tile_pool(name="pp", bufs=6))
    opool = ctx.enter_context(tc.tile_pool(name="ot", bufs=2))
    rpool = ctx.enter_context(tc.tile_pool(name="rec", bufs=8))
    # PSUM budget (8 banks): psproj 2 (separate QK/V slots, so the V matmuls
    # never wait on the DVE cast/shift readers of the QK slot) + psscore 2x2
    # + pspv 1 + pstr 2x128B (double-buffered transpose targets so the
    # PE-transpose -> DVE-copy chain pipelines instead of round-tripping).
    psproj = ctx.enter_context(tc.tile_pool(name="psproj", bufs=2, space="PSUM"))
    psscore = ctx.enter_context(tc.tile_pool(name="psscore", bufs=2, space="PSUM"))
    pspv = ctx.enter_context(tc.tile_pool(name="pspv", bufs=1, space="PSUM"))
    pstr = ctx.enter_context(tc.tile_pool(name="pstr", bufs=1, space="PSUM"))

    # ---- weights (dual queue: wqk on sync, wv on scalar) ----------------
    wqk_sb = persist.tile([P, EC, 2 * D], MMDT, tag="wqk")  # [Wq|Wk] packed
    wv_sb = persist.tile([P, EC, D], MMDT, tag="wv")
    nc.sync.dma_start(wqk_sb[:], wqk.rearrange("(c p) m -> p c m", p=P))
    nc.scalar.dma_start(wv_sb[:], wv.rearrange("(c p) m -> p c m", p=P))

    # ---- PE warm-up: keep HAM busy while the input streams in -----------
    warm_in = persist.tile([P, SBLK], MMDT, tag="warm")
    nc.vector.memset(warm_in[:], 0.25)
    # dummy activation: pull the ~1.3us ACT table load into the DMA phase
    warm_act = rpool.tile([P, 8], f32, tag="warmact")
    nc.scalar.activation(warm_act[:], warm_in[:, :8], AF.Exp, scale=1.0)
    # Small back-to-back matmuls spanning the whole x0 DMA wait (~4.5us):
    # HAM unthrottles after ~3.4us of sustained PE activity and, crucially,
    # does not re-throttle in the warmup->proj0 gap, so the latency-critical
    # ramp matmuls run at 2.4GHz instead of 1.2.
    wt = psproj.tile([P, SBLK], f32, tag="proj")
    for _ in range(44):
        nc.tensor.matmul(wt[:, :P], warm_in[:, :P], warm_in[:, :P], start=True, stop=True)

    # ---- constants -------------------------------------------------------
    ident = persist.tile([P, P], f32, tag="ident")
    nc.gpsimd.memset(ident[:], 0.0)
    nc.gpsimd.affine_select(
        out=ident[:], in_=ident[:],
        compare_op=mybir.AluOpType.not_equal, fill=1.0,
        base=0, pattern=[[-1, P]], channel_multiplier=1,
    )
    ident16 = persist.tile([P, P], MMDT, tag="ident16")
    nc.vector.tensor_copy(ident16[:], ident[:])

    # causal step mask: maskW[jj, c] = 1 iff c >= jj + SBLK
    maskW = persist.tile([P, 2 * SBLK], f32, tag="maskw")
    nc.gpsimd.memset(maskW[:], 1.0)
    nc.gpsimd.affine_select(
        out=maskW[:], in_=maskW[:],
        compare_op=mybir.AluOpType.is_ge, fill=0.0,
        base=-SBLK, pattern=[[1, 2 * SBLK]], channel_multiplier=-1,
    )
    mask16 = persist.tile([P, 2 * SBLK], MMDT, tag="mask16")
    nc.vector.tensor_copy(mask16[:], maskW[:])

    # ---- persistent activations -----------------------------------------
    # qk: rows 0:64 = Q^T, rows 64:128 = K^T (straight from packed psum)
    qk = persist.tile([P, S], MMDT, tag="qk")
    # partition-shifted copies (SBUF->SBUF DMA): K^T at rows 0:64, Q^T at 64:128
    kTlo = persist.tile([D, S], MMDT, tag="ktlo")
    qThi = persist.tile([P, S], MMDT, tag="qthi")  # rows 64:128 used
    vT = persist.tile([D, S], MMDT, tag="vt")
    vAug = persist.tile([P, NJT, 2 * D], MMDT, tag="vaug")
    yT = persist.tile([D, S], f32, tag="ytout")
    ones_f32 = persist.tile([P, NJT, D], f32, tag="ones")
    nc.vector.memset(ones_f32[:], 1.0)
    nc.vector.tensor_copy(vAug[:, :, D:], ones_f32[:])

    # ---- stream x^T block-by-block -------------------------------------
    # Static DMAs land on separate logical queues and would all stream
    # CONCURRENTLY from t=0, sharing bandwidth fairly and starving block 0.
    # Gate block b's load on earlier compute via tiny WAW writes into the
    # destination tile: the DMA then fires only once the gate ran, keeping
    # HBM focused on the block the pipeline needs next.
    xt_r = xt.rearrange("(c p) s -> p c s", p=P)
    xts = []
    for b in range(NSB):
        xts_b = xpool.tile([P, EC, SBLK], MMDT, tag=f"xts{b}", name=f"xts{b}")
        xts.append(xts_b)

    def stream_x(b):
        sl_b = slice(b * SBLK, (b + 1) * SBLK)
        nc.sync.dma_start(xts[b][:, : EC // 2, :], xt_r[:, : EC // 2, sl_b])
        nc.scalar.dma_start(xts[b][:, EC // 2 :, :], xt_r[:, EC // 2 :, sl_b])

    def gate_x(b, src):
        nc.vector.tensor_copy(xts[b][0:1, 0, 0:1], src)
        nc.vector.tensor_copy(xts[b][0:1, EC // 2, 0:1], src)

    def stream_x_gated(b):
        # SWDGE (gpsimd-generated) DMA: unlike static-queue HWDGE DMAs,
        # which all fire at t=0 and share bandwidth fairly, the SWDGE
        # trigger honors its sem waits, so the gate really delays the
        # transfer and HBM stays focused on the block the pipeline needs.
        sl_b = slice(b * SBLK, (b + 1) * SBLK)
        nc.gpsimd.dma_start(xts[b][:, : EC // 2, :], xt_r[:, : EC // 2, sl_b])
        nc.gpsimd.dma_start(xts[b][:, EC // 2 :, :], xt_r[:, EC // 2 :, sl_b])

    stream_x(0)
    gate_x(1, wt[0:1, 0:1])  # x1 fires once the warmup matmuls finish
    stream_x_gated(1)

    def qk_chunks(b):
        """Emit-steps for s-block b's Q/K projection."""
        sl = slice(b * SBLK, (b + 1) * SBLK)
        steps = []
        psQK = psproj.tile([P, SBLK], f32, tag="proj")

        def qk_mms(e0, psQK=psQK):
            for e in (e0, e0 + 1):
                nc.tensor.matmul(
                    psQK[:], wqk_sb[:, e, :], xts[b][:, e, :],
                    start=(e == 0), stop=(e == EC - 1),
                )

        for e0 in range(0, EC, 2):
            steps.append(lambda e0=e0: qk_mms(e0))

        def qk_out():
            nc.vector.tensor_copy(qk[:, sl], psQK[:])

        # partition-shifted copies on DVE (APs with equal partition counts
        # may live at different base partitions); f16 source for 2x rate
        def qk_shift_k():
            nc.vector.tensor_copy(kTlo[:, sl], qk[D:P, sl])

        def qk_shift_q():
            nc.vector.tensor_copy(qThi[D:P, sl], qk[:D, sl])

        steps.append(qk_out)
        steps.append(qk_shift_k)
        steps.append(qk_shift_q)
        if 1 <= b < NSB - 1:
            def gate_next():
                # release block b+1's x stream once this block's projection
                # is through the PE
                gate_x(b + 1, psQK[0:1, 0:1])
                stream_x_gated(b + 1)

            steps.append(gate_next)
        return steps

    def v_chunks(b):
        """Emit-steps for s-block b's V projection."""
        sl = slice(b * SBLK, (b + 1) * SBLK)
        steps = []
        psV = psproj.tile([P, SBLK], f32, tag="proj")

        def v_mms(e0, psV=psV):
            for e in (e0, e0 + 1):
                nc.tensor.matmul(
                    psV[:D, :], wv_sb[:, e, :], xts[b][:, e, :],
                    start=(e == 0), stop=(e == EC - 1),
                )

        for e0 in range(0, EC, 2):
            steps.append(lambda e0=e0: v_mms(e0))

        def v_out():
            nc.vector.tensor_copy(vT[:, sl], psV[:D, :])

        steps.append(v_out)

        # one two-half transpose target per block: alternating halves give
        # double-buffered PE-transpose -> DVE-copy pipelining in one bank
        psv2 = pstr.tile([P, 2, D], MMDT, tag="tr")

        def v_tr(t):
            j = 4 * b + t
            nc.tensor.transpose(
                psv2[:, t % 2, :], vT[:, j * P : (j + 1) * P], ident16[:D, :D]
            )
            nc.vector.tensor_copy(vAug[:, j, :D], psv2[:, t % 2, :])

        for t in range(4):
            steps.append(lambda t=t: v_tr(t))
        return steps

    def proj_chunks(b):
        return qk_chunks(b) + v_chunks(b)

    def scores_pair_for(b, pi):
        """Scores for kv tiles (2pi, 2pi+1) against q-block b. The second
        diagonal pair (tiles t=2,3) narrows BOTH members to cols >= 2P so
        its exp and mask each collapse to one strided instruction; all other
        pairs run full width. Narrowed matmuls still cover the full exp-read
        region, so exp never reads stale psum."""
        j0, j1 = 2 * pi, 2 * pi + 1
        o0 = 2 * P if (j0 - 4 * b) == 2 else 0
        o1 = 3 * P if (j1 - 4 * b) == 3 else 0
        ps = psscore.tile([P, 2, SBLK], f32, tag="score")
        q0 = slice(b * SBLK + o0, (b + 1) * SBLK)
        q1 = slice(b * SBLK + o1, (b + 1) * SBLK)
        # two PE row-groups: rows 0:64 (kTlo/qk) and 64:128 (qk/qThi)
        nc.tensor.matmul(
            ps[:, 0, o0:], kTlo[:, j0 * P : (j0 + 1) * P], qk[:D, q0],
        )
        nc.tensor.matmul(
            ps[:, 1, o1:], qk[D:P, j1 * P : (j1 + 1) * P], qThi[D:P, q1],
        )
        return (j0, j1, ps, (o0, o1))

    def attn(b, bg=(), inflight=None):
        bg = list(bg)
        nj = 4 * b + 4
        psO = pspv.tile([P, SBLK], f32, tag="pv")
        npairs = nj // 2
        if inflight is None:
            inflight = scores_pair_for(b, 0)
        nxt = None
        for pi in range(npairs):
            j0, j1, ps, offs = inflight
            pt = ppool.tile([P, 2, SBLK], MMDT, tag="pt")
            if offs == (0, 0):
                # fully-computed pair: one contiguous exp over both banks
                nc.scalar.activation(pt[:], ps[:], AF.Exp, scale=float(scale))
            else:
                # diagonal t2/t3 pair: exp only the causally-reachable columns
                for k, off in enumerate(offs):
                    nc.scalar.activation(
                        pt[:, k, off:], ps[:, k, off:], AF.Exp,
                        scale=float(scale),
                    )
            for k, j in enumerate((j0, j1)):
                t = j - 4 * b
                if t >= 0:
                    off = offs[k]
                    nc.vector.tensor_mul(
                        pt[:, k, off:],
                        pt[:, k, off:],
                        mask16[:, SBLK - t * P + off : 2 * SBLK - t * P],
                    )
            if pi + 1 < npairs:
                inflight = scores_pair_for(b, pi + 1)
            elif b + 1 < NSB:
                # cross-block prefetch: next block's first score pair goes
                # out before this block's last PV, so ScalarE rolls into the
                # next block's exps with no boundary bubble. Requires bg
                # (incl. next block's qk_out/shifts) fully drained by now.
                nxt = scores_pair_for(b + 1, 0)
            # independent next-block projection work keeps the PE busy
            # while this pair's exp runs on ScalarE; drain everything
            # before the last pair so the prefetched scores see their data
            if pi < npairs - 1:
                take = -(-len(bg) // (npairs - 1 - pi))
            else:
                take = len(bg)
            for _ in range(take):
                if bg:
                    bg.pop(0)()
            for k, j in enumerate((j0, j1)):
                off = offs[k]
                nc.tensor.matmul(
                    psO[:, off:], vAug[:, j, :],
                    pt[:, k, off:],
                    start=(j == 0), stop=(j == nj - 1),
                )
        return psO, nxt

    def out(b, psO):
        # rows 64:128 of psO hold the softmax denominator, pre-broadcast.
        # 1/s as exp(-ln s) on ScalarE: same ACT table set as the softmax exp,
        # and ~3x faster than DVE reciprocal at this shape. Two column-halves
        # pipeline ACT -> DVE -> DMA and release the PV bank sooner.
        sl = slice(b * SBLK, (b + 1) * SBLK)
        lns = rpool.tile([D, SBLK], f32, tag="lns")
        nc.scalar.activation(lns[:], psO[D:P, :], AF.Ln)
        rcp = rpool.tile([D, SBLK], f32, tag="rcp")
        nc.scalar.activation(rcp[:], lns[:], AF.Exp, scale=-1.0)
        nc.vector.tensor_mul(yT[:, sl], psO[:D, :], rcp[:])
        if b == NSB - 1:
            # tail-critical store: halves on both HWDGE rings overlap receipts
            h0 = slice(b * SBLK, b * SBLK + SBLK // 2)
            h1 = slice(b * SBLK + SBLK // 2, (b + 1) * SBLK)
            nc.sync.dma_start(y[:, h0], yT[:, h0])
            nc.scalar.dma_start(y[:, h1], yT[:, h1])
        else:
            nc.sync.dma_start(y[:, sl], yT[:, sl])

    # Software pipeline: block b's attention interleaves block b+1's
    # projection matmuls between its exp-gated pairs, so the PE never
    # drains while ScalarE (the critical engine) streams exps. Block b-1's
    # output normalization rides as the first background step, placing its
    # ScalarE recip between this block's exps instead of at the boundary.
    # Block 0: emit only the Q/K projection up front; its V projection
    # rides as the first background steps of attn(0). The pair-0 PV only
    # consumes vAug tiles 0..1, which the bg pops emit before it.
    for step in qk_chunks(0):
        step()
    prev, nxt = attn(0, bg=v_chunks(0) + proj_chunks(1))
    for b in range(1, NSB):
        bg = [lambda p=prev, bb=b: out(bb - 1, p)]
        if b + 1 < NSB:
            bg += proj_chunks(b + 1)
        prev, nxt = attn(b, bg=bg, inflight=nxt)
    out(NSB - 1, prev)


def build_nc():
    from contextlib import ExitStack

    _patch_tile_drain()
    nc = bass.Bass(target_bir_lowering=False, enable_partition_id=False)
    xt = nc.dram_tensor("xt", [E, S], MMDT, kind="ExternalInput")
    wqk = nc.dram_tensor("wqk", [E, 2 * D], MMDT, kind="ExternalInput")
    wv = nc.dram_tensor("wv", [E, D], MMDT, kind="ExternalInput")
    y = nc.dram_tensor("y", [D, S], f32, kind="ExternalOutput")
    with tile.TileContext(nc) as tc:
        with ExitStack() as ctx:
            _attention(ctx, tc, xt, wqk, wv, y)
    return nc


def make_in_maps(x, Wq, Wk, Wv):
    wqk = np.ascontiguousarray(np.concatenate([Wq, Wk], axis=1), dtype=MMNP)
    wv_c = np.ascontiguousarray(Wv, dtype=MMNP)
    x = np.asarray(x)
    return [
        {
            "xt": np.ascontiguousarray(x[b].T.astype(MMNP)),
            "wqk": wqk,
            "wv": wv_c,
        }
        for b in range(B)
    ]


_NC = None


def kernel(x, Wq, Wk, Wv, _trace=False, _tmpdir=None):
    from concourse.bass_utils import run_bass_kernel_spmd

    global _NC
    if _NC is None:
        _NC = build_nc()
        _split_multiwaits(_NC)  # walrus-only legalization; breaks CoreSim
    in_maps = make_in_maps(x, Wq, Wk, Wv)
    res = run_bass_kernel_spmd(
        _NC, in_maps, core_ids=list(range(B)), trace=_trace, tmpdir=_tmpdir
    )
    out = np.ascontiguousarray(
        np.stack([r["y"].T for r in res.results], axis=0), dtype=np.float32
    )
    if _trace:
        kernel.last_results = res
    return out

